# revision 4
# baseline (speedup 1.0000x reference)
"""Trainium2 Bass kernel for CellConditionedCrossContextAttn.

Math (per batch):
  a = masked_ln(x_a); b = masked_ln(x_b)
  C = layer_norm((cell_vec @ Wctx + bctx).reshape(LC, D))
  out_a = LN_masked(tri_attn(q=a, kv=b, C) + x_a)
  out_b = LN_masked(tri_attn(q=b, kv=a, C) + x_b)
where tri_attn scores[q,k,c] = <q_h[q], k_h[k] * c_h[c]> / sqrt(dh), softmax
over (k,c) jointly, out = sum_{k,c} attn * v[k].

Device mapping (8 NeuronCores, data-parallel over batch, 4 batches/core):
  - scores computed transposed per (head, c): S[k, (c,q)] = kc_c^T q, with kc
    built on DVE as k ⊙ c (per-partition scalar multiply in the [d, k] layout).
  - masking folded in as a rank-1 additive term (-226·invalid_k) ⊗ valid_q via
    a K=1 matmul accumulated into the same PSUM; dead-q columns are zeroed at
    the q-projection source, which reproduces the reference's uniform-softmax
    behaviour on fully-masked rows exactly.
  - exp on ScalarE straight out of PSUM (1/sqrt(dh) folded into the scale).
  - v-contraction with E stationary: out[q, 0:32] = unnormalized attn·v, and
    column 32 accumulates Z (softmax denominator) via an appended ones-column.
  - normalization = per-partition tensor_scalar multiply by 1/Z.
  - LN rstd via exp(-0.5·ln(var+eps)): the whole kernel uses one activation
    table set (natural_log_exp_and_others), no table switches.
"""

import contextlib
import os
import sys

for _p in ("/opt/trn_rl_repo", "/root/.axon_site/_ro/trn_rl_repo"):
    if os.path.isdir(_p) and _p not in sys.path:
        sys.path.insert(0, _p)

import numpy as np
import ml_dtypes

import concourse.bass as bass
import concourse.mybir as mybir
import concourse.tile as tile
from concourse.tile import ScopedClock
from concourse.bass_utils import run_bass_kernel_spmd

F32 = mybir.dt.float32
BF16 = mybir.dt.bfloat16
BF_NP = ml_dtypes.bfloat16

B, L, D, H, DH, LC, CIN = 32, 128, 256, 8, 32, 16, 512
NCORES = 8
BL = B // NCORES          # batches per core
EPS = 1e-5
SCALE = float(1.0 / np.sqrt(np.float32(DH)))
MASKVAL = -226.0          # * SCALE ~= -40  ->  exp() == 0 at fp32 resolution
AluOp = mybir.AluOpType
AF = mybir.ActivationFunctionType


# --------------------------------------------------------------------------
# TileContext patch: this walrus build accepts only ONE sync-wait command per
# instruction. Tile's scheduler may attach several; split the surplus onto
# same-engine NoOps emitted immediately before the instruction.
# --------------------------------------------------------------------------
_WAIT_LIMIT = 1
_split_n = [0]


def _split_waits(insts):
    new = []
    for inst in insts:
        si = inst.sync_info
        if si is not None and len(si.on_wait) > _WAIT_LIMIT:
            waits = list(si.on_wait)
            extra, keep = waits[:-_WAIT_LIMIT], waits[-_WAIT_LIMIT:]
            for w in extra:
                _split_n[0] += 1
                new.append(
                    mybir.InstNoOp(
                        name=f"I-wsplit-{_split_n[0]}",
                        engine=inst.engine,
                        ins=[],
                        outs=[],
                        sync_info=mybir.SyncInfo(on_wait=[w], on_update=[]),
                        bass_nofuse=True,
                    )
                )
            inst.sync_info = mybir.SyncInfo(on_wait=keep, on_update=list(si.on_update))
        new.append(inst)
    insts[:] = new


class PatchedTC(tile.TileContext):
    def _lower_ordered_insts(self, ordered):
        for insts in ordered.values():
            _split_waits(insts)
        return super()._lower_ordered_insts(ordered)

    def _drain_and_barrier(self, tick_clock, wait_clock):
        collector = self.nc.sync.nop(nofuse=True)
        wait_clock.add_sem_waits(
            collector.ins, ScopedClock({None: tick_clock.global_clock})
        )
        si = collector.ins.sync_info
        waits = list(si.on_wait) if si else []
        ups = list(si.on_update) if si else []
        if len(waits) > 1:
            collector.ins.sync_info = mybir.SyncInfo(on_wait=waits[:1], on_update=ups)
            for i in range(1, len(waits)):
                extra = self.nc.sync.nop(nofuse=True)
                extra.ins.sync_info = mybir.SyncInfo(on_wait=[waits[i]], on_update=[])
        self.nc.sync.drain()
        self.nc.all_engine_barrier()
        popped = self.nc._tile_sem_poison_stack.pop()
        assert popped is self._sem_poison
        self.nc.clear_and_free_semaphores(list(self.sems.allocated().values()))
        self.nc.all_engine_barrier()


# --------------------------------------------------------------------------
# Device program
# --------------------------------------------------------------------------

def _emit_ln(nc, sm, eps_t, out_bf, x_sb, vcol, rows):
    """LayerNorm over the free dim of x_sb[:rows] -> out_bf.

    With vcol (a [128,1] 0/1 fp32 AP) invalid rows pass through unchanged.
    rstd = exp(-0.5*ln(var+eps)) stays inside the ln/exp activation table set.
    """
    st6 = sm.tile([128, 6], F32, tag="st6")
    mv = sm.tile([128, 2], F32, tag="mv")
    nc.vector.bn_stats(st6[:rows], x_sb[:rows])
    nc.vector.bn_aggr(mv[:rows], st6[:rows])
    lnv = sm.tile([128, 1], F32, tag="sc")
    nc.scalar.activation(lnv[:rows], mv[:rows, 1:2], AF.Ln,
                         bias=eps_t[:rows], scale=1.0)
    rstd = sm.tile([128, 1], F32, tag="sc")
    nc.scalar.activation(rstd[:rows], lnv[:rows], AF.Exp, scale=-0.5)
    if vcol is not None:
        mu = sm.tile([128, 1], F32, tag="sc")
        nc.vector.tensor_scalar_mul(mu[:rows], mv[:rows, 0:1], vcol[:rows])
        t1 = sm.tile([128, 1], F32, tag="sc")
        nc.vector.tensor_scalar(t1[:rows], rstd[:rows], 1.0, vcol[:rows],
                                op0=AluOp.subtract, op1=AluOp.mult)
        rf = sm.tile([128, 1], F32, tag="sc")
        nc.vector.tensor_scalar_add(rf[:rows], t1[:rows], 1.0)
        mu_ap, rstd_ap = mu[:rows], rf[:rows]
    else:
        mu_ap, rstd_ap = mv[:rows, 0:1], rstd[:rows]
    nc.vector.tensor_scalar(out_bf[:rows], x_sb[:rows], mu_ap, rstd_ap,
                            op0=AluOp.subtract, op1=AluOp.mult)


def build_module():
    nc = bass.Bass("TRN2", target_bir_lowering=False, debug=False)

    # ---- DRAM I/O ----
    xa = nc.dram_tensor("xa", [BL, L, D], F32, kind="ExternalInput")
    xb = nc.dram_tensor("xb", [BL, L, D], F32, kind="ExternalInput")
    vcols = nc.dram_tensor("vcols", [BL, L, 2], F32, kind="ExternalInput")
    mrows = nc.dram_tensor("mrows", [BL, 2, 1, L], BF16, kind="ExternalInput")
    vqrows = nc.dram_tensor("vqrows", [BL, 2, 1, L], BF16, kind="ExternalInput")
    cellT16 = nc.dram_tensor("cellT16", [128, 4 * BL], BF16, kind="ExternalInput")
    wts = {}
    for wname in ("wq", "wk", "wv", "wc", "wo"):
        wts[wname] = nc.dram_tensor(wname, [128, 2 * D], BF16, kind="ExternalInput")
    wctx = nc.dram_tensor("wctx", [128, 4 * LC * D], BF16, kind="ExternalInput")
    bqc = nc.dram_tensor("bqc", [128, 2], F32, kind="ExternalInput")
    bkc = nc.dram_tensor("bkc", [128, 2], F32, kind="ExternalInput")
    bcc = nc.dram_tensor("bcc", [128, 2], F32, kind="ExternalInput")
    bvr = nc.dram_tensor("bvr", [1, D], BF16, kind="ExternalInput")
    bor = nc.dram_tensor("bor", [1, D], BF16, kind="ExternalInput")
    bctxr = nc.dram_tensor("bctxr", [1, LC * D], BF16, kind="ExternalInput")
    ya = nc.dram_tensor("ya", [BL, L, D], F32, kind="ExternalOutput")
    yb = nc.dram_tensor("yb", [BL, L, D], F32, kind="ExternalOutput")
    cscratch = nc.dram_tensor("cscratch", [BL, LC * D], F32)

    with PatchedTC(nc) as tc, contextlib.ExitStack() as ctx:
        st = ctx.enter_context(tc.tile_pool(name="static", bufs=1))
        xin = ctx.enter_context(tc.tile_pool(name="xin", bufs=6))
        lnp = ctx.enter_context(tc.tile_pool(name="lnp", bufs=4))
        tpp = ctx.enter_context(tc.tile_pool(name="tpp", bufs=4))
        prj = ctx.enter_context(tc.tile_pool(name="prj", bufs=4))
        aux = ctx.enter_context(tc.tile_pool(name="aux", bufs=3))
        kcp = ctx.enter_context(tc.tile_pool(name="kcp", bufs=6))
        ep = ctx.enter_context(tc.tile_pool(name="ep", bufs=10))
        op = ctx.enter_context(tc.tile_pool(name="op", bufs=3))
        fin = ctx.enter_context(tc.tile_pool(name="fin", bufs=4))
        sm = ctx.enter_context(tc.tile_pool(name="sm", bufs=8))
        Sp = ctx.enter_context(tc.tile_pool(name="Sp", bufs=2, space="PSUM"))
        vp = ctx.enter_context(tc.tile_pool(name="vp", bufs=2, space="PSUM"))
        pyp = ctx.enter_context(tc.tile_pool(name="pyp", bufs=2, space="PSUM"))

        # ---- constants / weights ----
        ones_row = st.tile([1, 128], BF16, tag="ones")
        nc.vector.memset(ones_row[:], 1.0)
        eps_t = st.tile([128, 1], F32, tag="eps")
        nc.vector.memset(eps_t[:], EPS)

        w_sb = {}
        for wname in ("wq", "wk", "wv", "wc", "wo"):
            w_sb[wname] = st.tile([128, 2 * D], BF16, tag=wname, name=wname)
            nc.sync.dma_start(w_sb[wname][:], wts[wname][:])
        wctx_sb = st.tile([128, 4 * LC * D], BF16, tag="wctx")
        nc.sync.dma_start(wctx_sb[:], wctx[:])
        bq_sb = st.tile([128, 2], F32, tag="bq")
        bk_sb = st.tile([128, 2], F32, tag="bk")
        bc_sb = st.tile([128, 2], F32, tag="bc")
        nc.sync.dma_start(bq_sb[:], bqc[:])
        nc.sync.dma_start(bk_sb[:], bkc[:])
        nc.sync.dma_start(bc_sb[:], bcc[:])
        bv_sb = st.tile([1, D], BF16, tag="bv")
        bo_sb = st.tile([1, D], BF16, tag="bo")
        bctx_sb = st.tile([1, LC * D], BF16, tag="bctx")
        nc.sync.dma_start(bv_sb[:], bvr[:])
        nc.sync.dma_start(bo_sb[:], bor[:])
        nc.sync.dma_start(bctx_sb[:], bctxr[:])
        vcol_sb = st.tile([128, 2 * BL], F32, tag="vcol")
        for b in range(BL):
            for s in range(2):
                nc.sync.dma_start(vcol_sb[:, 2 * b + s : 2 * b + s + 1],
                                  vcols[b, :, s : s + 1])
        cellT_sb = st.tile([128, 4 * BL], BF16, tag="cellT")
        nc.sync.dma_start(cellT_sb[:], cellT16[:])

        # ---- context tokens: C = cell @ Wctx + bctx -> LN -> cT ----
        C_sb = st.tile([BL, LC * D], F32, tag="Csb")
        for n in range(8):
            cp = pyp.tile([BL, 512], F32, tag="py")
            for k in range(4):
                nc.tensor.matmul(
                    cp[:], cellT_sb[:, 4 * k : 4 * k + BL],
                    wctx_sb[:, 4096 * k + 512 * n : 4096 * k + 512 * n + 512],
                    start=(k == 0), stop=False, skip_group_check=True)
            nc.tensor.matmul(cp[:], ones_row[0:1, 0:BL],
                             bctx_sb[:, 512 * n : 512 * n + 512],
                             start=False, stop=True, skip_group_check=True)
            nc.vector.tensor_copy(C_sb[:, 512 * n : 512 * n + 512], cp[:])
        nc.sync.dma_start(cscratch[:], C_sb[:])
        Cr = st.tile([BL * LC, D], F32, tag="Cr")
        nc.sync.dma_start(Cr[:], cscratch[:].rearrange("b (l d) -> (b l) d", l=LC))
        Cln = st.tile([BL * LC, D], BF16, tag="Cln")
        _emit_ln(nc, sm, eps_t, Cln, Cr, None, BL * LC)
        ClnT = st.tile([128, 2 * BL * LC], BF16, tag="ClnT")
        for kchunk in range(2):
            nc.sync.dma_start_transpose(
                ClnT[:, 64 * kchunk : 64 * kchunk + BL * LC],
                Cln[:, 128 * kchunk : 128 * kchunk + 128])
        cT_sb = st.tile([128, 2 * BL * LC], F32, tag="cT")
        for m in range(2):
            cps = pyp.tile([128, BL * LC], F32, tag="py")
            for kchunk in range(2):
                nc.tensor.matmul(
                    cps[:],
                    w_sb["wc"][:, 256 * kchunk + 128 * m : 256 * kchunk + 128 * m + 128],
                    ClnT[:, 64 * kchunk : 64 * kchunk + BL * LC],
                    start=(kchunk == 0), stop=(kchunk == 1), skip_group_check=True)
            nc.vector.tensor_scalar_add(cT_sb[:, 64 * m : 64 * m + BL * LC],
                                        cps[:], bc_sb[:, m : m + 1])

        # ---- per-batch pipeline ----
        for b in range(BL):
            va = vcol_sb[:, 2 * b : 2 * b + 1]
            vb = vcol_sb[:, 2 * b + 1 : 2 * b + 2]
            x_a = xin.tile([128, D], F32, tag="x")
            x_b = xin.tile([128, D], F32, tag="x")
            nc.sync.dma_start(x_a[:], xa[b])
            nc.sync.dma_start(x_b[:], xb[b])
            x_dir = (x_a, x_b)

            abf = lnp.tile([128, D], BF16, tag="lnbf")
            bbf = lnp.tile([128, D], BF16, tag="lnbf")
            _emit_ln(nc, sm, eps_t, abf, x_a, va, 128)
            _emit_ln(nc, sm, eps_t, bbf, x_b, vb, 128)
            azq = lnp.tile([128, D], BF16, tag="zq")
            bzq = lnp.tile([128, D], BF16, tag="zq")
            nc.vector.tensor_scalar_mul(azq[:], abf[:], va)
            nc.vector.tensor_scalar_mul(bzq[:], bbf[:], vb)

            aT = [tpp.tile([128, 2 * L], BF16, tag="aT", name="aT") for _ in range(2)]
            zT = [tpp.tile([128, 2 * L], BF16, tag="zT", name="zT") for _ in range(2)]
            for m in range(2):
                nc.sync.dma_start_transpose(aT[m][:, 0:128], abf[:, 128 * m : 128 * m + 128])
                nc.sync.dma_start_transpose(aT[m][:, 128:256], bbf[:, 128 * m : 128 * m + 128])
                nc.sync.dma_start_transpose(zT[m][:, 0:128], azq[:, 128 * m : 128 * m + 128])
                nc.sync.dma_start_transpose(zT[m][:, 128:256], bzq[:, 128 * m : 128 * m + 128])

            # transposed q/k projections: rows = dout chunk m, cols = (a|b)
            qTs, kTs, vext = {}, {}, {}
            for wname, rhs_tiles, dst, bias in (
                ("wq", zT, qTs, bq_sb),
                ("wk", aT, kTs, bk_sb),
            ):
                for m in range(2):
                    pp = pyp.tile([128, 2 * L], F32, tag="py")
                    for kchunk in range(2):
                        nc.tensor.matmul(
                            pp[:],
                            w_sb[wname][:, 256 * kchunk + 128 * m : 256 * kchunk + 128 * m + 128],
                            rhs_tiles[kchunk][:],
                            start=(kchunk == 0), stop=(kchunk == 1),
                            skip_group_check=True)
                    t = prj.tile([128, 2 * L], BF16, tag=wname + "Ts")
                    nc.vector.tensor_scalar_add(t[:], pp[:], bias[:, m : m + 1])
                    dst[m] = t

            # natural-layout v projections (+ ones column for Z)
            for side in range(2):  # 0: v from a, 1: v from b
                vps = pyp.tile([128, D], F32, tag="py")
                for kchunk in range(2):
                    nc.tensor.matmul(vps[:],
                                     aT[kchunk][:, 128 * side : 128 * side + 128],
                                     w_sb["wv"][:, 256 * kchunk : 256 * kchunk + 256],
                                     start=(kchunk == 0), stop=False,
                                     skip_group_check=True)
                nc.tensor.matmul(vps[:], ones_row[:], bv_sb[:],
                                 start=False, stop=True, skip_group_check=True)
                vx = prj.tile([128, H * (DH + 1)], BF16, tag="vext")
                nc.vector.memset(vx[:], 1.0)
                nc.vector.tensor_copy(
                    vx[:].rearrange("p (h c) -> p h c", c=DH + 1)[:, :, 0:DH],
                    vps[:].rearrange("p (h c) -> p h c", c=DH))
                vext[side] = vx

            # ---- attention for both directions of this batch ----
            for d in range(2):
                # d=0: out_a (q from a, k/v from b); d=1: out_b
                qcol = d
                kcol = 1 - d
                auxm = aux.tile([128, 128], BF16, tag="auxm")
                auxv = aux.tile([128, 128], BF16, tag="auxv")
                for r in (0, 32, 64, 96):
                    nc.sync.dma_start(auxm[r : r + 1, :], mrows[b, d])
                    nc.sync.dma_start(auxv[r : r + 1, :], vqrows[b, d])

                kc = {}
                for m in range(2):
                    for oct_ in range(2):
                        t = kcp.tile([128, 8 * L], BF16, tag="kc")
                        for cc in range(8):
                            c = 8 * oct_ + cc
                            nc.vector.tensor_scalar_mul(
                                t[:, 128 * cc : 128 * cc + 128],
                                kTs[m][:, 128 * kcol : 128 * kcol + 128],
                                cT_sb[:, 64 * m + LC * b + c : 64 * m + LC * b + c + 1])
                        kc[(m, oct_)] = t

                E_tiles = {}
                for oct_ in range(2):
                    for hp in range(4):  # head pairs (0,1),(2,3),(4,5),(6,7)
                        Ss = []
                        for h in (2 * hp, 2 * hp + 1):
                            m, s = h // 4, h % 4
                            S = Sp.tile([128, 8 * L], F32, tag="S")
                            for cc in range(8):
                                nc.tensor.matmul(
                                    S[:, 128 * cc : 128 * cc + 128],
                                    kc[(m, oct_)][32 * s : 32 * s + 32,
                                                  128 * cc : 128 * cc + 128],
                                    qTs[m][32 * s : 32 * s + 32,
                                           128 * qcol : 128 * qcol + 128],
                                    start=True, stop=False,
                                    tile_position=(32 * s, 0),
                                    skip_group_check=True)
                            Ss.append((h, S))
                        for h, S in Ss:
                            s = h % 4
                            for half in range(2):
                                nc.tensor.matmul(
                                    S[:, 512 * half : 512 * half + 512],
                                    auxm[32 * s : 32 * s + 1, :],
                                    auxv[32 * s : 32 * s + 1, :][:, None, :]
                                        .to_broadcast((1, 4, L)),
                                    start=False, stop=(half == 1),
                                    tile_position=(32 * s, 0),
                                    skip_group_check=True)
                            E = ep.tile([128, 8 * L], BF16, tag="E")
                            nc.scalar.activation(E[:], S[:], AF.Exp, scale=SCALE)
                            E_tiles[(h, oct_)] = E

                osb = op.tile([128, D], BF16, tag="osb")
                for h in range(H):
                    vo = vp.tile([128, DH + 1], F32, tag="vout")
                    for oct_ in range(2):
                        for cc in range(8):
                            nc.tensor.matmul(
                                vo[:],
                                E_tiles[(h, oct_)][:, 128 * cc : 128 * cc + 128],
                                vext[kcol][:, (DH + 1) * h : (DH + 1) * h + DH + 1],
                                start=(oct_ == 0 and cc == 0),
                                stop=(oct_ == 1 and cc == 7),
                                skip_group_check=True)
                    zc = sm.tile([128, 1], F32, tag="sc")
                    nc.vector.tensor_copy(zc[:], vo[:, DH : DH + 1])
                    rz = sm.tile([128, 1], F32, tag="sc")
                    nc.vector.reciprocal(rz[:], zc[:])
                    nc.vector.tensor_scalar_mul(
                        osb[:, DH * h : DH * h + DH], vo[:, 0:DH], rz[:])

                otT = op.tile([128, D], BF16, tag="otT")
                for m in range(2):
                    nc.sync.dma_start_transpose(
                        otT[:, 128 * m : 128 * m + 128],
                        osb[:, 128 * m : 128 * m + 128])
                yps = pyp.tile([128, D], F32, tag="py")
                for kchunk in range(2):
                    nc.tensor.matmul(yps[:], otT[:, 128 * kchunk : 128 * kchunk + 128],
                                     w_sb["wo"][:, 256 * kchunk : 256 * kchunk + 256],
                                     start=(kchunk == 0), stop=False,
                                     skip_group_check=True)
                nc.tensor.matmul(yps[:], ones_row[:], bo_sb[:],
                                 start=False, stop=True, skip_group_check=True)
                r_sb = fin.tile([128, D], F32, tag="r")
                nc.vector.tensor_tensor(r_sb[:], yps[:], x_dir[d][:], op=AluOp.add)
                yout = fin.tile([128, D], F32, tag="yout")
                _emit_ln(nc, sm, eps_t, yout, r_sb,
                         vcol_sb[:, 2 * b + d : 2 * b + d + 1], 128)
                nc.sync.dma_start((ya if d == 0 else yb)[b], yout[:])

    return nc


# --------------------------------------------------------------------------
# Host side
# --------------------------------------------------------------------------

def _prep_shared(params):
    p = params
    sh = {}

    def packw(w):
        w = np.asarray(w, np.float32)
        kchunks = w.shape[0] // 128
        return np.ascontiguousarray(
            w.reshape(kchunks, 128, w.shape[1]).transpose(1, 0, 2)
            .reshape(128, kchunks * w.shape[1])).astype(BF_NP)

    sh["wq"] = packw(p["Wq"]); sh["wk"] = packw(p["Wk"])
    sh["wv"] = packw(p["Wv"]); sh["wc"] = packw(p["Wc"]); sh["wo"] = packw(p["Wo"])
    sh["wctx"] = packw(p["Wctx"])
    sh["bqc"] = np.ascontiguousarray(np.asarray(p["bq"], np.float32).reshape(2, 128).T)
    sh["bkc"] = np.ascontiguousarray(np.asarray(p["bk"], np.float32).reshape(2, 128).T)
    sh["bcc"] = np.ascontiguousarray(np.asarray(p["bc"], np.float32).reshape(2, 128).T)
    sh["bvr"] = np.asarray(p["bv"], np.float32).reshape(1, D).astype(BF_NP)
    sh["bor"] = np.asarray(p["bo"], np.float32).reshape(1, D).astype(BF_NP)
    sh["bctxr"] = np.asarray(p["bctx"], np.float32).reshape(1, LC * D).astype(BF_NP)
    return sh


def _prep_core(ci, x_a, x_b, valid_a, valid_b, cell_vec, shared):
    s = slice(BL * ci, BL * ci + BL)
    va = np.asarray(valid_a[s], np.float32)   # [BL, L]
    vb = np.asarray(valid_b[s], np.float32)
    m = dict(shared)
    m["xa"] = np.ascontiguousarray(np.asarray(x_a[s], np.float32))
    m["xb"] = np.ascontiguousarray(np.asarray(x_b[s], np.float32))
    m["vcols"] = np.ascontiguousarray(np.stack([va, vb], axis=-1))   # [BL,L,2]
    # direction 0 (out_a): k side = b, q side = a; direction 1: swapped
    mr = np.stack([MASKVAL * (1.0 - vb), MASKVAL * (1.0 - va)], axis=1)
    vq = np.stack([va, vb], axis=1)
    m["mrows"] = np.ascontiguousarray(mr[:, :, None, :]).astype(BF_NP)
    m["vqrows"] = np.ascontiguousarray(vq[:, :, None, :]).astype(BF_NP)
    cT = np.asarray(cell_vec[s], np.float32).T            # [CIN, BL]
    m["cellT16"] = np.ascontiguousarray(
        cT.reshape(4, 128, BL).transpose(1, 0, 2).reshape(128, 4 * BL)).astype(BF_NP)
    return m


def _np_reference(x_a, x_b, valid_a, valid_b, cell_vec, params):
    """Plain numpy fallback (used only if structural assumptions fail)."""
    p = {k: np.asarray(v, np.float32) for k, v in params.items()}
    NEG = np.float32(-1e9)

    def ln(x, g, bb):
        mu = x.mean(-1, keepdims=True)
        var = ((x - mu) ** 2).mean(-1, keepdims=True)
        return (x - mu) / np.sqrt(var + EPS) * g + bb

    def mln(x, v, g, bb):
        return np.where(v[..., None], ln(x, g, bb), x)

    def sheads(x, W, bv):
        y = x @ W + bv
        return y.reshape(y.shape[0], y.shape[1], H, DH).transpose(0, 2, 1, 3)

    def tri(q_in, k_in, v_in, Cc, mask):
        q = sheads(q_in, p["Wq"], p["bq"]); k = sheads(k_in, p["Wk"], p["bk"])
        v = sheads(v_in, p["Wv"], p["bv"]); c = sheads(Cc, p["Wc"], p["bc"])
        sc = np.einsum("bhqd,bhkd,bhcd->bhqkc", q, k, c) / np.float32(np.sqrt(DH))
        sc = np.where(mask[:, None, :, :, None], NEG, sc)
        b_, h_, lq, lk, lc = sc.shape
        f = sc.reshape(b_, h_, lq, lk * lc)
        f = f - f.max(-1, keepdims=True)
        e = np.exp(f)
        attn = (e / e.sum(-1, keepdims=True)).reshape(b_, h_, lq, lk, lc)
        out = np.einsum("bhqkc,bhkd->bhqd", attn, v)
        out = out.transpose(0, 2, 1, 3).reshape(b_, lq, D)
        return out @ p["Wo"] + p["bo"]

    x_a = np.asarray(x_a, np.float32); x_b = np.asarray(x_b, np.float32)
    valid_a = np.asarray(valid_a, bool); valid_b = np.asarray(valid_b, bool)
    cell_vec = np.asarray(cell_vec, np.float32)
    mB2A = ~(valid_a[:, :, None] & valid_b[:, None, :])
    mA2B = ~(valid_b[:, :, None] & valid_a[:, None, :])
    a = mln(x_a, valid_a, p["ln_a_g"], p["ln_a_b"])
    bt = mln(x_b, valid_b, p["ln_b_g"], p["ln_b_b"])
    C = (cell_vec @ p["Wctx"] + p["bctx"]).reshape(B, LC, D)
    C = ln(C, p["ln_ctx_g"], p["ln_ctx_b"])
    oa = tri(a, bt, bt, C, mB2A)
    oa = mln(oa + x_a, valid_a, p["ln_out_a_g"], p["ln_out_a_b"])
    ob = tri(bt, a, a, C, mA2B)
    ob = mln(ob + x_b, valid_b, p["ln_out_b_g"], p["ln_out_b_b"])
    return oa, ob


_CACHE = {}


def _get_nc():
    if "nc" not in _CACHE:
        _CACHE["nc"] = build_module()
    return _CACHE["nc"]


def _structural_ok(x_a, x_b, valid_a, valid_b, cell_vec, params):
    try:
        if tuple(np.shape(x_a)) != (B, L, D): return False
        if tuple(np.shape(x_b)) != (B, L, D): return False
        if tuple(np.shape(valid_a)) != (B, L): return False
        if tuple(np.shape(valid_b)) != (B, L): return False
        if tuple(np.shape(cell_vec)) != (B, CIN): return False
        for g in ("ln_a_g", "ln_b_g", "ln_out_a_g", "ln_out_b_g", "ln_ctx_g"):
            if not np.allclose(np.asarray(params[g]), 1.0): return False
        for bnm in ("ln_a_b", "ln_b_b", "ln_out_a_b", "ln_out_b_b", "ln_ctx_b"):
            if not np.allclose(np.asarray(params[bnm]), 0.0): return False
        return True
    except Exception:
        return False


def make_in_maps(x_a, x_b, valid_a, valid_b, cell_vec, params):
    shared = _prep_shared(params)
    return [
        _prep_core(ci, x_a, x_b, valid_a, valid_b, cell_vec, shared)
        for ci in range(NCORES)
    ]


def assemble(results):
    out_a = np.concatenate([r["ya"] for r in results], axis=0)
    out_b = np.concatenate([r["yb"] for r in results], axis=0)
    return out_a, out_b


def kernel(x_a, x_b, valid_a, valid_b, cell_vec, params):
    if not _structural_ok(x_a, x_b, valid_a, valid_b, cell_vec, params):
        return _np_reference(x_a, x_b, valid_a, valid_b, cell_vec, params)
    nc = _get_nc()
    in_maps = make_in_maps(x_a, x_b, valid_a, valid_b, cell_vec, params)
    res = run_bass_kernel_spmd(nc, in_maps, core_ids=list(range(NCORES)))
    return assemble(res.results)
